# revision 9
# baseline (speedup 1.0000x reference)
"""Trainium2 Bass kernel: DiscreteEmbedding (rect-window embedding lookup).

Math (matches the jax reference bitwise):
    xs  = x * 2048
    out[t] = 0.5*T[ceil(xs+0.5)-1] + 0.5*T[floor(xs+0.5)]
where T is extended with zero rows for index 2048 (xs >= 2047.5 tail).
For non-boundary tokens both indices coincide -> exactly T[round(xs)];
for exact-half xs the two halves blend, identical to the sign-window matmul.

Strategy (8 cores, data-parallel over tokens):
  - host: split 65536 tokens into 8 shards of 8192; per core pass x wrapped
    as [16, 512] and replicated to [128, 512] (all index math runs full-width,
    and partitions 16..127 double as the replicated index copies the SWDGE
    gather hardware expects).
  - device: fp32 index math on DVE -> int16 index buffers; build a 0.5-scaled,
    zero-extended table copy in DRAM; two chunked dma_gather passes (lo/hi);
    DVE add; contiguous 1MB stores of position-ordered rows.
  - host: un-permute rows (position order -> token order), concat shards.
"""

import numpy as np

import concourse.bass as bass
import concourse.mybir as mybir
import concourse.tile as tile
from concourse import bacc, bass_utils

N_CORES = 8
B, S = 32, 2048
V, D = 2048, 128
TOK = B * S                 # 65536 tokens total
TPC = TOK // N_CORES        # 8192 tokens per core
SPC = TPC // 16             # 512: free dim of the wrapped [16, 512] x layout
NCH = 4                     # gather pipeline chunks per core
CW = SPC // NCH             # 128 idx columns per chunk
JB = TPC // 128 // NCH      # 16 j-blocks (128-row groups) per chunk
VEXT = V + 128              # table rows incl. zero rows (indices reach 2048)

F32 = mybir.dt.float32
I32 = mybir.dt.int32
I16 = mybir.dt.int16
OP = mybir.AluOpType


def build():
    nc = bacc.Bacc("TRN2", target_bir_lowering=False, debug=False, num_devices=N_CORES)
    xr = nc.dram_tensor("xr", [128, SPC], F32, kind="ExternalInput")
    emb = nc.dram_tensor("emb", [V, D], F32, kind="ExternalInput")
    out = nc.dram_tensor("out", [TPC, D], F32, kind="ExternalOutput")
    tbl_half = nc.dram_tensor("tbl_half", [VEXT, D], F32, kind="Internal")

    with tile.TileContext(nc) as tc:
        with tc.tile_pool(name="sb", bufs=1) as sb, tc.tile_pool(name="g", bufs=2) as gp:
            # ---- halved + zero-extended table copy in DRAM ----
            tbl = sb.tile([128, (V // 128) * D], F32)
            nc.sync.dma_start(
                out=tbl[:], in_=emb[:].rearrange("(p n) d -> p (n d)", p=128)
            )
            nc.vector.tensor_scalar_mul(tbl[:], tbl[:], 0.5)
            nc.sync.dma_start(
                out=tbl_half[0:V].rearrange("(p n) d -> p (n d)", p=128), in_=tbl[:]
            )
            zt = sb.tile([128, D], F32)
            nc.vector.memset(zt[:], 0.0)
            nc.sync.dma_start(
                out=tbl_half[V:VEXT].rearrange("(p n) d -> p (n d)", p=128), in_=zt[:]
            )

            # ---- index math (fp32, exact): y = x*2048 + 0.5 ----
            xt = sb.tile([128, SPC], F32)
            nc.sync.dma_start(out=xt[:], in_=xr[:])
            y = sb.tile([128, SPC], F32)
            nc.vector.tensor_scalar(y[:], xt[:], 2048.0, 0.5, op0=OP.mult, op1=OP.add)
            # i0 = int(y) rounded to SOME neighboring integer; correct to
            # floor/ceil with exact fp32 compares (robust to HW round mode).
            i0 = sb.tile([128, SPC], I32)
            nc.vector.tensor_copy(i0[:], y[:])
            f0 = sb.tile([128, SPC], F32)
            nc.vector.tensor_copy(f0[:], i0[:])
            gt = sb.tile([128, SPC], F32)
            nc.vector.tensor_tensor(gt[:], f0[:], y[:], op=OP.is_gt)
            lt = sb.tile([128, SPC], F32)
            nc.vector.tensor_tensor(lt[:], f0[:], y[:], op=OP.is_lt)
            hf = sb.tile([128, SPC], F32)   # floor(y)
            nc.vector.tensor_sub(hf[:], f0[:], gt[:])
            lf = sb.tile([128, SPC], F32)   # ceil(y) - 1
            nc.vector.tensor_add(lf[:], f0[:], lt[:])
            nc.vector.tensor_scalar_add(lf[:], lf[:], -1.0)
            hi16 = sb.tile([128, SPC], I16)
            nc.vector.tensor_copy(hi16[:], hf[:])
            lo16 = sb.tile([128, SPC], I16)
            nc.vector.tensor_copy(lo16[:], lf[:])

            # ---- chunked dual gather + add + store ----
            out_v = out[:].rearrange("(p j) d -> p (j d)", p=128)
            for ch in range(NCH):
                glo = gp.tile([128, JB * D], F32, tag="glo")
                ghi = gp.tile([128, JB * D], F32, tag="ghi")
                nc.gpsimd.dma_gather(
                    glo[:].rearrange("p (j d) -> p j d", d=D),
                    tbl_half[:],
                    lo16[:, ch * CW : (ch + 1) * CW],
                    num_idxs=128 * JB,
                    num_idxs_reg=128 * JB,
                    elem_size=D,
                    single_packet=False,
                )
                nc.gpsimd.dma_gather(
                    ghi[:].rearrange("p (j d) -> p j d", d=D),
                    tbl_half[:],
                    hi16[:, ch * CW : (ch + 1) * CW],
                    num_idxs=128 * JB,
                    num_idxs_reg=128 * JB,
                    elem_size=D,
                    single_packet=False,
                )
                nc.vector.tensor_add(glo[:], glo[:], ghi[:])
                nc.sync.dma_start(
                    out=out_v[:, ch * JB * D : (ch + 1) * JB * D], in_=glo[:]
                )
    nc.compile()
    return nc


_NC = None


def _row_perm():
    """out_dram row r holds gather position i(r); position i handles token
    t(i) = (i%16)*512 + i//16 (x wrapped [16,512] across partitions)."""
    r = np.arange(TPC)
    p, j = r // 64, r % 64
    i = (j // JB) * (128 * JB) + (j % JB) * 128 + p
    return (i % 16) * SPC + i // 16  # token index held at row r


def kernel(x, time_embedding):
    global _NC
    x = np.ascontiguousarray(np.asarray(x, dtype=np.float32))
    t = np.ascontiguousarray(np.asarray(time_embedding, dtype=np.float32))
    xf = x.reshape(-1)
    in_maps = []
    for c in range(N_CORES):
        xc = xf[c * TPC : (c + 1) * TPC].reshape(16, SPC)
        in_maps.append({"xr": np.ascontiguousarray(np.tile(xc, (8, 1))), "emb": t})

    if _NC is None:
        _NC = build()
    res = bass_utils.run_bass_kernel_spmd(_NC, in_maps, core_ids=list(range(N_CORES)))
    global _LAST_RES
    _LAST_RES = res

    tkn = _row_perm()
    outs = []
    for c in range(N_CORES):
        oc = np.asarray(res.results[c]["out"])
        full = np.empty_like(oc)
        full[tkn] = oc
        outs.append(full)
    return np.concatenate(outs, axis=0).reshape(B, S, D)


# revision 11
# speedup vs baseline: 1.3216x; 1.3216x over previous
"""Trainium2 Bass kernel: DiscreteEmbedding (rect-window embedding lookup).

Math (matches the jax reference bitwise):
    xs  = x * 2048;  y = xs + 0.5
    i_lo = ceil(y)-1, i_hi = floor(y)
    out[t] = 0.5*T[i_lo] + 0.5*T[i_hi]      (T extended with zero row 2048)
Non-boundary tokens (y non-integer): i_lo == i_hi -> out = T[i_lo].
Boundary tokens (y integer, ~1/4096 of tokens): out = avg of two rows.

Device strategy (8 cores, data-parallel over tokens):
  - Build a combined table TC in DRAM:
      TC[0:2048]    = T            (plain rows)
      TC[2048]      = 0            (i_lo == 2048 tail -> zero output)
      TC[2049+k]    = (T[k]+T[k+1])/2  for k<2048, with T[2048]=0
    and gather ONCE per token at idx2 = i_lo + 2049*b, b = (y integer).
    This halves GPSIMD descriptor-generation work vs a dual gather - the
    measured bottleneck (~10 ns/idx on one SWDGE queue).
  - 4 SWDGE queues, gather chunks round-robin -> parallel Q7 desc-gen.
  - x is passed wrapped [16,512] replicated to [128,512]: full-width DVE
    index math, and partitions 16..127 double as the per-Q7-core replicas
    of the int16 index buffer that dma_gather expects.
  - Gather output is position-ordered; stores are contiguous 1MB DMAs;
    host un-permutes rows (free) while un-sharding.
"""

import numpy as np

import concourse.bass as bass
import concourse.mybir as mybir
import concourse.tile as tile
from concourse import bacc, bass_utils

N_CORES = 8
B, S = 32, 2048
V, D = 2048, 128
TOK = B * S                 # 65536 tokens total
TPC = TOK // N_CORES        # 8192 tokens per core
SPC = TPC // 16             # 512: free dim of the wrapped [16, 512] x layout
NCH = 4                     # gather pipeline chunks per core
CW = SPC // NCH             # idx columns per chunk
JB = TPC // 128 // NCH      # j-blocks (128-row groups) per chunk
ABASE = V + 1               # 2049: base row of the averaged-pair table
VEXT = 4224                 # TC rows (>= 2*V+1, multiple of 128)
NQ = 4                      # SWDGE queues

F32 = mybir.dt.float32
I32 = mybir.dt.int32
I16 = mybir.dt.int16
OP = mybir.AluOpType


def build():
    nc = bacc.Bacc(
        "TRN2",
        target_bir_lowering=False,
        debug=False,
        num_devices=N_CORES,
        num_swdge_queues=NQ,
    )
    xr = nc.dram_tensor("xr", [128, SPC], F32, kind="ExternalInput")
    emb = nc.dram_tensor("emb", [V, D], F32, kind="ExternalInput")
    out = nc.dram_tensor("out", [TPC, D], F32, kind="ExternalOutput")
    tc_tbl = nc.dram_tensor("tc_tbl", [VEXT, D], F32, kind="Internal")

    with tile.TileContext(nc) as tc:
        with tc.tile_pool(name="sb", bufs=1) as sb, tc.tile_pool(name="g", bufs=3) as gp:
            # ---- combined table TC = [T; 0; avg-pairs] ----
            tbl = sb.tile([128, (V // 128) * D], F32)   # T rows, 16 rows/partition
            nc.sync.dma_start(
                out=tbl[:], in_=emb[:].rearrange("(p n) d -> p (n d)", p=128)
            )
            # shifted table: ssh row k = T[k+1], with T[2048] = 0
            ssh = sb.tile([128, (V // 128) * D], F32)
            nc.vector.memset(ssh[:], 0.0)
            nc.sync.dma_start(
                out=ssh[0:127, :],
                in_=emb[1 : 1 + 127 * 16].rearrange("(p n) d -> p (n d)", p=127),
            )
            nc.sync.dma_start(
                out=ssh[127:128, 0 : 15 * D],
                in_=emb[127 * 16 + 1 : V].rearrange("(p n) d -> p (n d)", p=1),
            )
            avg = sb.tile([128, (V // 128) * D], F32)
            nc.vector.tensor_add(avg[:], tbl[:], ssh[:])
            nc.vector.tensor_scalar_mul(avg[:], avg[:], 0.5)
            zrow = sb.tile([1, D], F32)
            nc.vector.memset(zrow[:], 0.0)
            nc.sync.dma_start(
                out=tc_tbl[0:V].rearrange("(p n) d -> p (n d)", p=128), in_=tbl[:]
            )
            nc.sync.dma_start(out=tc_tbl[V : V + 1, :], in_=zrow[:])
            nc.sync.dma_start(
                out=tc_tbl[ABASE : ABASE + V].rearrange("(p n) d -> p (n d)", p=128),
                in_=avg[:],
            )

            # ---- index math (fp32, exact): y = x*2048 + 0.5 ----
            xt = sb.tile([128, SPC], F32)
            nc.sync.dma_start(out=xt[:], in_=xr[:])
            y = sb.tile([128, SPC], F32)
            nc.vector.tensor_scalar(y[:], xt[:], 2048.0, 0.5, op0=OP.mult, op1=OP.add)
            # i0 = int(y) rounded to SOME neighboring integer; fix up with
            # exact fp32 compares (robust to the HW float->int round mode).
            i0 = sb.tile([128, SPC], I32)
            nc.vector.tensor_copy(i0[:], y[:])
            f0 = sb.tile([128, SPC], F32)
            nc.vector.tensor_copy(f0[:], i0[:])
            lt = sb.tile([128, SPC], F32)    # f0 < y
            nc.vector.tensor_tensor(lt[:], f0[:], y[:], op=OP.is_lt)
            bnd = sb.tile([128, SPC], F32)   # y integer -> blend row
            nc.vector.tensor_tensor(bnd[:], f0[:], y[:], op=OP.is_equal)
            lf = sb.tile([128, SPC], F32)    # i_lo = ceil(y) - 1
            nc.vector.tensor_add(lf[:], f0[:], lt[:])
            nc.vector.tensor_scalar_add(lf[:], lf[:], -1.0)
            # idx2 = i_lo + 2049*b
            idxf = sb.tile([128, SPC], F32)
            nc.vector.scalar_tensor_tensor(
                out=idxf[:],
                in0=bnd[:],
                scalar=float(ABASE),
                in1=lf[:],
                op0=OP.mult,
                op1=OP.add,
            )
            idx16 = sb.tile([128, SPC], I16)
            nc.vector.tensor_copy(idx16[:], idxf[:])

            # ---- chunked gather + store ----
            out_v = out[:].rearrange("(p j) d -> p (j d)", p=128)
            for ch in range(NCH):
                g = gp.tile([128, JB * D], F32, tag="g")
                nc.gpsimd.dma_gather(
                    g[:].rearrange("p (j d) -> p j d", d=D),
                    tc_tbl[0 : ABASE + V],
                    idx16[:, ch * CW : (ch + 1) * CW],
                    num_idxs=128 * JB,
                    num_idxs_reg=128 * JB,
                    elem_size=D,
                    single_packet=False,
                    queue_num=ch % NQ,
                )
                nc.sync.dma_start(
                    out=out_v[:, ch * JB * D : (ch + 1) * JB * D], in_=g[:]
                )
    nc.compile()
    return nc


_NC = None


def _row_perm():
    """out row r holds gather position i(r); position i handles token
    t(i) = (i%16)*512 + i//16 (x wrapped [16,512] across partitions)."""
    r = np.arange(TPC)
    p, j = r // 64, r % 64
    i = (j // JB) * (128 * JB) + (j % JB) * 128 + p
    return (i % 16) * SPC + i // 16  # token index held at row r


def kernel(x, time_embedding):
    global _NC
    x = np.ascontiguousarray(np.asarray(x, dtype=np.float32))
    t = np.ascontiguousarray(np.asarray(time_embedding, dtype=np.float32))
    xf = x.reshape(-1)
    in_maps = []
    for c in range(N_CORES):
        xc = xf[c * TPC : (c + 1) * TPC].reshape(16, SPC)
        in_maps.append({"xr": np.ascontiguousarray(np.tile(xc, (8, 1))), "emb": t})

    if _NC is None:
        _NC = build()
    res = bass_utils.run_bass_kernel_spmd(_NC, in_maps, core_ids=list(range(N_CORES)))
    global _LAST_RES
    _LAST_RES = res

    tkn = _row_perm()
    outs = []
    for c in range(N_CORES):
        oc = np.asarray(res.results[c]["out"])
        full = np.empty_like(oc)
        full[tkn] = oc
        outs.append(full)
    return np.concatenate(outs, axis=0).reshape(B, S, D)


# revision 12
# speedup vs baseline: 2.0245x; 1.5318x over previous
"""Trainium2 Bass kernel: DiscreteEmbedding (rect-window embedding lookup).

Math (matches the jax reference bitwise):
    xs  = x * 2048;  y = xs + 0.5
    i_lo = ceil(y)-1, i_hi = floor(y)
    out[t] = 0.5*T[i_lo] + 0.5*T[i_hi]      (T extended with zero row 2048)
Non-boundary tokens (y non-integer): i_lo == i_hi -> out = T[i_lo].
Boundary tokens (y integer, ~1/4096 of tokens): out = avg of two rows.

Device strategy (8 cores, data-parallel over tokens):
  - Build a combined table TC in DRAM:
      TC[0:2048]    = T            (plain rows)
      TC[2048]      = 0            (i_lo == 2048 tail -> zero output)
      TC[2049+k]    = (T[k]+T[k+1])/2  for k<2048, with T[2048]=0
    and gather ONCE per token at idx2 = i_lo + 2049*b, b = (y integer).
    This halves GPSIMD descriptor-generation work vs a dual gather - the
    measured bottleneck (~10 ns/idx on one SWDGE queue).
  - 4 SWDGE queues, gather chunks round-robin -> parallel Q7 desc-gen.
  - x is passed wrapped [16,512] replicated to [128,512]: full-width DVE
    index math, and partitions 16..127 double as the per-Q7-core replicas
    of the int16 index buffer that dma_gather expects.
  - Gather output is position-ordered; stores are contiguous 1MB DMAs;
    host un-permutes rows (free) while un-sharding.
"""

import numpy as np

import concourse.bass as bass
import concourse.mybir as mybir
import concourse.tile as tile
from concourse import bacc, bass_utils

N_CORES = 8
B, S = 32, 2048
V, D = 2048, 128
TOK = B * S                 # 65536 tokens total
TPC = TOK // N_CORES        # 8192 tokens per core
SPC = TPC // 16             # 512: free dim of the wrapped [16, 512] x layout
NCH = 4                     # gather pipeline chunks per core
CW = SPC // NCH             # idx columns per chunk
JB = TPC // 128 // NCH      # j-blocks (128-row groups) per chunk
ABASE = V + 1               # 2049: base row of the averaged-pair table
VEXT = 4224                 # TC rows (>= 2*V+1, multiple of 128)
NQ = 4                      # SWDGE queues

F32 = mybir.dt.float32
I32 = mybir.dt.int32
I16 = mybir.dt.int16
OP = mybir.AluOpType


def build():
    nc = bacc.Bacc(
        "TRN2",
        target_bir_lowering=False,
        debug=False,
        num_devices=N_CORES,
        num_swdge_queues=NQ,
    )
    xr = nc.dram_tensor("xr", [128, SPC], F32, kind="ExternalInput")
    emb = nc.dram_tensor("emb", [V, D], F32, kind="ExternalInput")
    out = nc.dram_tensor("out", [TPC, D], F32, kind="ExternalOutput")
    tc_tbl = nc.dram_tensor("tc_tbl", [VEXT, D], F32, kind="Internal")

    with tile.TileContext(nc) as tc:
        with tc.tile_pool(name="sb", bufs=1) as sb, tc.tile_pool(name="g", bufs=NCH) as gp:
            # ---- x load first so index math runs during table prep ----
            xt = sb.tile([128, SPC], F32)
            nc.sync.dma_start(out=xt[:], in_=xr[:])

            # ---- combined table TC = [T; 0; avg-pairs] ----
            tbl = sb.tile([128, (V // 128) * D], F32)   # T rows, 16 rows/partition
            nc.sync.dma_start(
                out=tbl[:], in_=emb[:].rearrange("(p n) d -> p (n d)", p=128)
            )
            # avg[k] = (T[k]+T[k+1])/2.  Within a partition (rows 16p..16p+15)
            # the +1 shift is a free-dim offset; the n=15 element needs the
            # next partition's first row (tnext), fetched by a tiny
            # SBUF->SBUF partition-shifted DMA; last partition pads zero.
            tnext = sb.tile([128, D], F32)
            nc.vector.memset(tnext[:], 0.0)
            nc.sync.dma_start(out=tnext[0:127, :], in_=tbl[1:128, 0:D])
            avg = sb.tile([128, (V // 128) * D], F32)
            nc.vector.tensor_add(
                avg[:, 0 : 15 * D], tbl[:, 0 : 15 * D], tbl[:, D : 16 * D]
            )
            nc.vector.tensor_add(avg[:, 15 * D : 16 * D], tbl[:, 15 * D : 16 * D], tnext[:])
            nc.vector.tensor_scalar_mul(avg[:], avg[:], 0.5)
            zrow = sb.tile([1, D], F32)
            nc.vector.memset(zrow[:], 0.0)
            nc.scalar.dma_start(
                out=tc_tbl[0:V].rearrange("(p n) d -> p (n d)", p=128), in_=tbl[:]
            )
            nc.scalar.dma_start(out=tc_tbl[V : V + 1, :], in_=zrow[:])
            nc.scalar.dma_start(
                out=tc_tbl[ABASE : ABASE + V].rearrange("(p n) d -> p (n d)", p=128),
                in_=avg[:],
            )

            # ---- index math (fp32, exact): y = x*2048 + 0.5 ----
            y = sb.tile([128, SPC], F32)
            nc.vector.tensor_scalar(y[:], xt[:], 2048.0, 0.5, op0=OP.mult, op1=OP.add)
            # i0 = int(y) rounded to SOME neighboring integer; fix up with
            # exact fp32 compares (robust to the HW float->int round mode).
            i0 = sb.tile([128, SPC], I32)
            nc.vector.tensor_copy(i0[:], y[:])
            f0 = sb.tile([128, SPC], F32)
            nc.vector.tensor_copy(f0[:], i0[:])
            lt = sb.tile([128, SPC], F32)    # f0 < y
            nc.vector.tensor_tensor(lt[:], f0[:], y[:], op=OP.is_lt)
            bnd = sb.tile([128, SPC], F32)   # y integer -> blend row
            nc.vector.tensor_tensor(bnd[:], f0[:], y[:], op=OP.is_equal)
            lf = sb.tile([128, SPC], F32)    # i_lo = ceil(y) - 1
            nc.vector.tensor_add(lf[:], f0[:], lt[:])
            nc.vector.tensor_scalar_add(lf[:], lf[:], -1.0)
            # idx2 = i_lo + 2049*b
            idxf = sb.tile([128, SPC], F32)
            nc.vector.scalar_tensor_tensor(
                out=idxf[:],
                in0=bnd[:],
                scalar=float(ABASE),
                in1=lf[:],
                op0=OP.mult,
                op1=OP.add,
            )
            idx16 = sb.tile([128, SPC], I16)
            nc.vector.tensor_copy(idx16[:], idxf[:])

            # ---- chunked gather + store ----
            out_v = out[:].rearrange("(p j) d -> p (j d)", p=128)
            for ch in range(NCH):
                g = gp.tile([128, JB * D], F32, tag="g")
                nc.gpsimd.dma_gather(
                    g[:].rearrange("p (j d) -> p j d", d=D),
                    tc_tbl[0 : ABASE + V],
                    idx16[:, ch * CW : (ch + 1) * CW],
                    num_idxs=128 * JB,
                    num_idxs_reg=128 * JB,
                    elem_size=D,
                    single_packet=False,
                    queue_num=ch % NQ,
                )
                nc.sync.dma_start(
                    out=out_v[:, ch * JB * D : (ch + 1) * JB * D], in_=g[:]
                )
    nc.compile()
    return nc


_NC = None


def _row_perm():
    """out row r holds gather position i(r); position i handles token
    t(i) = (i%16)*512 + i//16 (x wrapped [16,512] across partitions)."""
    r = np.arange(TPC)
    p, j = r // 64, r % 64
    i = (j // JB) * (128 * JB) + (j % JB) * 128 + p
    return (i % 16) * SPC + i // 16  # token index held at row r


def kernel(x, time_embedding):
    global _NC
    x = np.ascontiguousarray(np.asarray(x, dtype=np.float32))
    t = np.ascontiguousarray(np.asarray(time_embedding, dtype=np.float32))
    xf = x.reshape(-1)
    in_maps = []
    for c in range(N_CORES):
        xc = xf[c * TPC : (c + 1) * TPC].reshape(16, SPC)
        in_maps.append({"xr": np.ascontiguousarray(np.tile(xc, (8, 1))), "emb": t})

    if _NC is None:
        _NC = build()
    res = bass_utils.run_bass_kernel_spmd(_NC, in_maps, core_ids=list(range(N_CORES)))
    global _LAST_RES
    _LAST_RES = res

    tkn = _row_perm()
    outs = []
    for c in range(N_CORES):
        oc = np.asarray(res.results[c]["out"])
        full = np.empty_like(oc)
        full[tkn] = oc
        outs.append(full)
    return np.concatenate(outs, axis=0).reshape(B, S, D)


# revision 13
# speedup vs baseline: 2.2564x; 1.1145x over previous
"""Trainium2 Bass kernel: DiscreteEmbedding (rect-window embedding lookup).

Math (matches the jax reference bitwise):
    xs  = x * 2048;  y = xs + 0.5
    i_lo = ceil(y)-1, i_hi = floor(y)
    out[t] = 0.5*T[i_lo] + 0.5*T[i_hi]      (T extended with zero row 2048)
Non-boundary tokens (y non-integer): i_lo == i_hi -> out = T[i_lo].
Boundary tokens (y integer, ~1/4096 of tokens): out = avg of two rows.

Device strategy (8 cores, data-parallel over tokens):
  - Build a combined table TC in DRAM:
      TC[0:2048]    = T            (plain rows)
      TC[2048]      = 0            (i_lo == 2048 tail -> zero output)
      TC[2049+k]    = (T[k]+T[k+1])/2  for k<2048, with T[2048]=0
    and gather ONCE per token at idx2 = i_lo + 2049*b, b = (y integer).
    This halves GPSIMD descriptor-generation work vs a dual gather - the
    measured bottleneck (~10 ns/idx on one SWDGE queue).
  - 4 SWDGE queues, gather chunks round-robin -> parallel Q7 desc-gen.
  - x is passed wrapped [16,512] replicated to [128,512]: full-width DVE
    index math, and partitions 16..127 double as the per-Q7-core replicas
    of the int16 index buffer that dma_gather expects.
  - Gather output is position-ordered; stores are contiguous 1MB DMAs;
    host un-permutes rows (free) while un-sharding.
"""

import numpy as np

import concourse.bass as bass
import concourse.mybir as mybir
import concourse.tile as tile
from concourse import bacc, bass_utils

N_CORES = 8
B, S = 32, 2048
V, D = 2048, 128
TOK = B * S                 # 65536 tokens total
TPC = TOK // N_CORES        # 8192 tokens per core
SPC = TPC // 16             # 512: free dim of the wrapped [16, 512] x layout
NCH = 4                     # gather pipeline chunks per core
CW = SPC // NCH             # idx columns per chunk
JB = TPC // 128 // NCH      # j-blocks (128-row groups) per chunk
ABASE = V + 1               # 2049: base row of the averaged-pair table
VEXT = 4224                 # TC rows (>= 2*V+1, multiple of 128)
NQ = 4                      # SWDGE queues

F32 = mybir.dt.float32
I32 = mybir.dt.int32
I16 = mybir.dt.int16
OP = mybir.AluOpType


def build():
    nc = bacc.Bacc(
        "TRN2",
        target_bir_lowering=False,
        debug=False,
        num_devices=N_CORES,
        num_swdge_queues=NQ,
    )
    xr = nc.dram_tensor("xr", [128, SPC], F32, kind="ExternalInput")
    emb = nc.dram_tensor("emb", [V, D], F32, kind="ExternalInput")
    out = nc.dram_tensor("out", [TPC, D], F32, kind="ExternalOutput")
    tc_tbl = nc.dram_tensor("tc_tbl", [VEXT, D], F32, kind="Internal")

    with tile.TileContext(nc) as tc:
        with tc.tile_pool(name="sb", bufs=1) as sb, tc.tile_pool(name="g", bufs=NCH) as gp:
            # ---- x load first so index math runs during table prep ----
            xt = sb.tile([128, SPC], F32)
            nc.sync.dma_start(out=xt[:], in_=xr[:])

            # ---- combined table TC = [T; 0; avg-pairs] ----
            tbl = sb.tile([128, (V // 128) * D], F32)   # T rows, 16 rows/partition
            nc.sync.dma_start(
                out=tbl[:], in_=emb[:].rearrange("(p n) d -> p (n d)", p=128)
            )
            # avg[k] = (T[k]+T[k+1])/2.  Within a partition (rows 16p..16p+15)
            # the +1 shift is a free-dim offset; the n=15 element needs the
            # next partition's first row (tnext), fetched by a tiny
            # SBUF->SBUF partition-shifted DMA; last partition pads zero.
            tnext = sb.tile([128, D], F32)
            nc.vector.memset(tnext[:], 0.0)
            # tnext[p] = T[16(p+1)] straight from DRAM (strided rows 16,32,..2032)
            nc.sync.dma_start(
                out=tnext[0:127, :],
                in_=emb[16:V].rearrange("(p n) d -> p (n d)", p=127)[:, 0:D],
            )
            avg = sb.tile([128, (V // 128) * D], F32)
            nc.vector.tensor_add(
                avg[:, 0 : 15 * D], tbl[:, 0 : 15 * D], tbl[:, D : 16 * D]
            )
            nc.vector.tensor_add(avg[:, 15 * D : 16 * D], tbl[:, 15 * D : 16 * D], tnext[:])
            nc.vector.tensor_scalar_mul(avg[:], avg[:], 0.5)
            zrow = sb.tile([1, D], F32)
            nc.vector.memset(zrow[:], 0.0)
            nc.scalar.dma_start(
                out=tc_tbl[0:V].rearrange("(p n) d -> p (n d)", p=128), in_=tbl[:]
            )
            nc.scalar.dma_start(out=tc_tbl[V : V + 1, :], in_=zrow[:])
            nc.scalar.dma_start(
                out=tc_tbl[ABASE : ABASE + V].rearrange("(p n) d -> p (n d)", p=128),
                in_=avg[:],
            )

            # ---- index math (fp32, exact): y = x*2048 + 0.5 ----
            y = sb.tile([128, SPC], F32)
            nc.vector.tensor_scalar(y[:], xt[:], 2048.0, 0.5, op0=OP.mult, op1=OP.add)
            # i0 = int(y) rounded to SOME neighboring integer; fix up with
            # exact fp32 compares (robust to the HW float->int round mode).
            i0 = sb.tile([128, SPC], I32)
            nc.vector.tensor_copy(i0[:], y[:])
            f0 = sb.tile([128, SPC], F32)
            nc.vector.tensor_copy(f0[:], i0[:])
            lt = sb.tile([128, SPC], F32)    # f0 < y
            nc.vector.tensor_tensor(lt[:], f0[:], y[:], op=OP.is_lt)
            bnd = sb.tile([128, SPC], F32)   # y integer -> blend row
            nc.vector.tensor_tensor(bnd[:], f0[:], y[:], op=OP.is_equal)
            lf = sb.tile([128, SPC], F32)    # i_lo = ceil(y) - 1
            nc.vector.tensor_add(lf[:], f0[:], lt[:])
            nc.vector.tensor_scalar_add(lf[:], lf[:], -1.0)
            # idx2 = i_lo + 2049*b
            idxf = sb.tile([128, SPC], F32)
            nc.vector.scalar_tensor_tensor(
                out=idxf[:],
                in0=bnd[:],
                scalar=float(ABASE),
                in1=lf[:],
                op0=OP.mult,
                op1=OP.add,
            )
            idx16 = sb.tile([128, SPC], I16)
            nc.vector.tensor_copy(idx16[:], idxf[:])

            # ---- chunked gather + store ----
            out_v = out[:].rearrange("(p j) d -> p (j d)", p=128)
            for ch in range(NCH):
                g = gp.tile([128, JB * D], F32, tag="g")
                nc.gpsimd.dma_gather(
                    g[:].rearrange("p (j d) -> p j d", d=D),
                    tc_tbl[0 : ABASE + V],
                    idx16[:, ch * CW : (ch + 1) * CW],
                    num_idxs=128 * JB,
                    num_idxs_reg=128 * JB,
                    elem_size=D,
                    single_packet=False,
                    queue_num=ch % NQ,
                )
                nc.sync.dma_start(
                    out=out_v[:, ch * JB * D : (ch + 1) * JB * D], in_=g[:]
                )
    nc.compile()
    return nc


_NC = None


def _row_perm():
    """out row r holds gather position i(r); position i handles token
    t(i) = (i%16)*512 + i//16 (x wrapped [16,512] across partitions)."""
    r = np.arange(TPC)
    p, j = r // 64, r % 64
    i = (j // JB) * (128 * JB) + (j % JB) * 128 + p
    return (i % 16) * SPC + i // 16  # token index held at row r


def kernel(x, time_embedding):
    global _NC
    x = np.ascontiguousarray(np.asarray(x, dtype=np.float32))
    t = np.ascontiguousarray(np.asarray(time_embedding, dtype=np.float32))
    xf = x.reshape(-1)
    in_maps = []
    for c in range(N_CORES):
        xc = xf[c * TPC : (c + 1) * TPC].reshape(16, SPC)
        in_maps.append({"xr": np.ascontiguousarray(np.tile(xc, (8, 1))), "emb": t})

    if _NC is None:
        _NC = build()
    res = bass_utils.run_bass_kernel_spmd(_NC, in_maps, core_ids=list(range(N_CORES)))
    global _LAST_RES
    _LAST_RES = res

    tkn = _row_perm()
    outs = []
    for c in range(N_CORES):
        oc = np.asarray(res.results[c]["out"])
        full = np.empty_like(oc)
        full[tkn] = oc
        outs.append(full)
    return np.concatenate(outs, axis=0).reshape(B, S, D)
